# revision 14
# baseline (speedup 1.0000x reference)
"""Causal self-attention on 8 Trainium2 NeuronCores.

Problem: B=4, T=2048, C=1024, H=16, DH=64.
  qkv = x @ w_qkv.T ; causal softmax attention per head ; y = attnout @ w_out.T

Sharding: 8 cores = 4 batches x 2 query-subsets. Each core computes the full
QKV projection for its batch (duplicated within the pair -> no collectives),
then attention for a load-balanced set of query rows (all 16 heads), then
the output projection for its own query rows against the full w_out.

Query balance under causality: global 512-row q-tiles are paired (i, 3-i):
  parity 0 -> q512 tiles [0, 3] (20 key-tiles), parity 1 -> [1, 2] (20).

Everything runs in "transposed space": Q^T/K^T are produced head-pair-stacked
[128=2x64 dh rows, T], scores are computed as S^T (keys on PSUM partitions,
two heads concurrently via PE row-tiling), the softmax denominator is
accumulated on the PE itself (all-ones stationary), PV produces attnout^T
directly (two heads via PE column-tiling), and the output projection consumes
attnout^T as its stationary operand — no transposes in any inner loop.

Schedule: phase-separated with targeted overlap. x^T and the w_v/w_out
transposes stream first, then the K/Q projections (PE-saturated), then just
enough V projection for the first attention q-tile; the remaining V tiles
are interleaved with the j=0 attention, and the j=0 output projection is
interleaved at pair boundaries of the j=1 attention. Diagonal k-tiles slice
scores/exp/PV to the causally-live q-range, the softmax reciprocal uses the
fast approx DVE op (via an SBUF staging copy — the custom DVE op mangles
PSUM reads on HW), and transpose results leave PSUM four blocks per copy.
"""

import threading

import numpy as np

B, T, C = 4, 2048, 1024
H = 16
DH = C // H
P = 128
TL = T // 2          # query rows per core
NPAIR = H // 2       # 8 head-pairs
NCT = C // P         # 8 c-tiles
QT_TILE = 512        # q columns per attention tile
NQT = TL // QT_TILE  # 2 local q-tiles

# local q512-tile -> global q512-tile, per parity (also the Q-proj map)
QMAP512 = [[0, 3], [1, 2]]

_cache = {}


def _build_program(parity: int):
    import concourse.mybir as mybir
    import concourse.tile as tile
    from concourse import bacc
    from concourse.masks import make_identity

    f32 = mybir.dt.float32
    bf16 = mybir.dt.bfloat16

    nc = bacc.Bacc("TRN2", target_bir_lowering=False, debug=False)
    x = nc.dram_tensor("x", [T, C], f32, kind="ExternalInput").ap()
    w_qkv = nc.dram_tensor("w_qkv", [3 * C, C], f32, kind="ExternalInput").ap()
    w_out = nc.dram_tensor("w_out", [C, C], f32, kind="ExternalInput").ap()
    y = nc.dram_tensor("y", [TL, C], f32, kind="ExternalOutput").ap()

    g512 = QMAP512[parity]
    scale = 1.0 / float(np.sqrt(DH))

    with tile.TileContext(nc) as tc:
        with (
            tc.tile_pool(name="res", bufs=1) as res,
            tc.tile_pool(name="stage", bufs=3) as stage,
            tc.tile_pool(name="wtile", bufs=2) as wtile,
            tc.tile_pool(name="work", bufs=2) as work,
            tc.tile_pool(name="rdp", bufs=2) as rdp,
            tc.tile_pool(name="attn", bufs=2) as attnp,
        ):
            ones128 = res.tile([P, P], bf16)
            nc.vector.memset(ones128, 1.0)

            # [128,128] triangle mask: keep (1.0) iff q-col >= key-row
            tri = res.tile([P, P], bf16)
            nc.gpsimd.memset(tri, 1.0)
            nc.gpsimd.affine_select(
                out=tri, in_=tri, compare_op=mybir.AluOpType.is_ge,
                fill=0.0, base=0, pattern=[[1, P]], channel_multiplier=-1,
            )

            ident = res.tile([P, P], bf16)
            make_identity(nc, ident)

            # ---- residents
            kT = res.tile([P, NPAIR, T], bf16)          # K^T   4 MB
            qT = res.tile([P, NPAIR, TL], bf16)         # Q^T   2 MB
            v = res.tile([P, T // P, C], bf16)          # V     4 MB
            wvT = res.tile([P, NCT, C], bf16)           # w_v^T 2 MB
            woT = res.tile([P, NCT, C], bf16)           # w_out^T 2 MB
            xT = res.tile([P, NCT, T], bf16)            # x^T   4 MB

            def rr_copy(out, in_):
                # Scalar/ACT is the exp-pacing engine — keep every copy off
                # it and on the (lighter) Vector engine.
                nc.vector.tensor_copy(out=out, in_=in_)

            def load_cast(src_ap):
                lf = stage.tile([P, C], f32, tag="ldf")
                nc.sync.dma_start(out=lf, in_=src_ap)
                lb = stage.tile([P, C], bf16, tag="ldb")
                nc.vector.tensor_copy(out=lb, in_=lf)
                return lb

            def make_transpose_block(pool):
                def transpose_block(lb, dst, dst_col):
                    for half in range(2):
                        pt4 = pool.tile([P, 4, P], bf16, tag="pt4",
                                        name="pt4")
                        for q in range(4):
                            ct = half * 4 + q
                            nc.tensor.transpose(
                                pt4[:, q, :], lb[:, ct * P:(ct + 1) * P],
                                ident)
                        rr_copy(
                            out=dst[:, half * 4:(half + 1) * 4,
                                    dst_col:dst_col + P],
                            in_=pt4)
                return transpose_block

            # ====== phases A+B: transposes + K/Q projections ======
            with (
                tc.tile_pool(name="pst", bufs=2, space="PSUM") as pst,
                tc.tile_pool(name="psqkv", bufs=2, space="PSUM") as psqkv,
            ):
                transpose_block = make_transpose_block(pst)

                # A: x^T, interleaved with the w_v / w_out block transposes
                for tt in range(T // P):
                    xb = load_cast(x[tt * P:(tt + 1) * P, :])
                    transpose_block(xb, xT, tt * P)
                    if tt < 8:
                        wb = load_cast(w_qkv[(16 + tt) * P:(17 + tt) * P, :])
                        transpose_block(wb, wvT, tt * P)
                    else:
                        fb = tt - 8
                        wb = load_cast(w_out[fb * P:(fb + 1) * P, :])
                        transpose_block(wb, woT, fb * P)

                # B: Q/K projections (Q: only local halves)
                for fb in range(16):                     # 0..7 Q, 8..15 K
                    wb = load_cast(w_qkv[fb * P:(fb + 1) * P, :])
                    wqk = wtile.tile([P, NCT, P], bf16, tag="wqk")
                    transpose_block(wb, wqk, 0)
                    # two 512-col accumulation groups per PSUM tile, copied
                    # out [128,1024] at a time (halves the copy/sem count)
                    if fb < 8:
                        ps = psqkv.tile([P, 2, 512], f32, tag="psqkv")
                        for u in range(NQT):
                            t0 = g512[u] * 512
                            for ct in range(NCT):
                                nc.tensor.matmul(
                                    ps[:, u, :], wqk[:, ct, :],
                                    xT[:, ct, t0:t0 + 512],
                                    start=(ct == 0), stop=(ct == NCT - 1),
                                )
                        nc.vector.tensor_copy(out=qT[:, fb, :], in_=ps)
                    else:
                        pr = fb - 8
                        for uu in range(0, 4, 2):
                            ps = psqkv.tile([P, 2, 512], f32, tag="psqkv")
                            for h in range(2):
                                u = uu + h
                                for ct in range(NCT):
                                    nc.tensor.matmul(
                                        ps[:, h, :], wqk[:, ct, :],
                                        xT[:, ct, u * 512:(u + 1) * 512],
                                        start=(ct == 0), stop=(ct == NCT - 1),
                                    )
                            nc.vector.tensor_copy(
                                out=kT[:, pr, uu * 512:(uu + 2) * 512],
                                in_=ps)

            # ====== phases C-F: V projection + attention + out-proj ======
            with (
                tc.tile_pool(name="psv", bufs=2, space="PSUM") as psv,
                tc.tile_pool(name="pss", bufs=2, space="PSUM") as pss,
                tc.tile_pool(name="pso", bufs=2, space="PSUM") as pso,
            ):
                def v_step(tt, fo):
                    ps = psv.tile([P, 512], f32, tag="psv")
                    for ct in range(NCT):
                        nc.tensor.matmul(
                            ps, xT[:, ct, tt * P:(tt + 1) * P],
                            wvT[:, ct, fo * 512:(fo + 1) * 512],
                            start=(ct == 0), stop=(ct == NCT - 1),
                        )
                    nc.vector.tensor_copy(
                        out=v[:, tt, fo * 512:(fo + 1) * 512], in_=ps)

                attnT = [
                    attnp.tile([P, NPAIR, QT_TILE], bf16, tag="attnT",
                               name=f"attnT{j}")
                    for j in range(NQT)
                ]

                def attn_pair_steps(p, j):
                    """List of emission steps for attention of (pair, qtile).
                    Scores for k-tile k are emitted together with the
                    softmax+PV of k-tile k-1 so the PE never queues directly
                    behind the ACT engine's exp."""
                    G = g512[j]
                    nk = 4 * (G + 1)
                    box = {}
                    qA = qT[0:64, p, j * QT_TILE:(j + 1) * QT_TILE]
                    qB = qT[64:128, p, j * QT_TILE:(j + 1) * QT_TILE]

                    def softmax_pv(k, s2, qlo):
                        dj = k - 4 * G
                        p2 = work.tile([P, 2, QT_TILE], bf16, tag="p2")
                        nc.scalar.activation(
                            p2[:, :, qlo:], s2[:, :, qlo:],
                            mybir.ActivationFunctionType.Exp, scale=scale)
                        if dj >= 0:  # triangle block on the diagonal
                            nc.vector.tensor_mul(
                                out=p2[:, :, qlo:qlo + P],
                                in0=p2[:, :, qlo:qlo + P],
                                in1=tri[:, None, :].to_broadcast((P, 2, P)))
                        pA = p2[:, 0, qlo:]
                        pB = p2[:, 1, qlo:]
                        st, sp_ = (k == 0), (k == nk - 1)
                        nc.tensor.matmul(box["pdA"][:, qlo:],
                                         ones128[:, 0:64], pA,
                                         start=st, stop=sp_,
                                         skip_group_check=True)
                        nc.tensor.matmul(box["pdB"][:, qlo:],
                                         ones128[:, 0:64], pB,
                                         start=st, stop=sp_,
                                         skip_group_check=True)
                        nc.tensor.matmul(
                            box["poA"][:, qlo:], v[:, k, p * P:p * P + 64],
                            pA, start=st, stop=sp_, skip_group_check=True)
                        nc.tensor.matmul(
                            box["poB"][:, qlo:],
                            v[:, k, p * P + 64:(p + 1) * P], pB,
                            start=st, stop=sp_, skip_group_check=True)

                    steps = []

                    def mk_ktile(k):
                        dj = k - 4 * G
                        qlo = max(dj, 0) * P

                        def s():
                            if k == 0:
                                poA_t = pso.tile([P, QT_TILE], f32,
                                                 tag="po", name="poA")
                                poB_t = pso.tile([P, QT_TILE], f32,
                                                 tag="po", name="poB")
                                box["poA"] = poA_t[0:64]
                                box["poB"] = poB_t[64:128]
                                box["pdA"] = poB_t[0:64]
                                box["pdB"] = poA_t[64:128]
                            s2 = pss.tile([P, 2, QT_TILE], f32, tag="s2")
                            ks = slice(k * P, (k + 1) * P)
                            nc.tensor.matmul(s2[:, 0, qlo:],
                                             kT[0:64, p, ks], qA[:, qlo:])
                            nc.tensor.matmul(s2[:, 1, qlo:],
                                             kT[64:128, p, ks], qB[:, qlo:])
                            if box.get("pending") is not None:
                                softmax_pv(*box["pending"])
                            box["pending"] = (k, s2, qlo)
                        return s
                    steps += [mk_ktile(k) for k in range(nk)]

                    def s_finish():
                        softmax_pv(*box["pending"])
                        # reciprocal_approx_fast mangles PSUM reads on HW —
                        # stage the denominators through SBUF first.
                        dcp = rdp.tile([P, QT_TILE], f32, tag="dcp")
                        nc.vector.tensor_copy(out=dcp[0:64], in_=box["pdA"])
                        nc.vector.tensor_copy(out=dcp[64:128],
                                              in_=box["pdB"])
                        rD = rdp.tile([P, QT_TILE], f32, tag="rD")
                        nc.vector.reciprocal_approx_fast(rD, dcp)
                        nc.vector.tensor_mul(
                            out=attnT[j][0:64, p, :], in0=box["poA"],
                            in1=rD[0:64])
                        nc.vector.tensor_mul(
                            out=attnT[j][64:128, p, :], in0=box["poB"],
                            in1=rD[64:128])
                    steps.append(s_finish)
                    return steps

                def outproj_chunk(j, sub, fo):
                    qs = slice(sub * P, (sub + 1) * P)
                    ps = pso.tile([P, 512], f32, tag="po", name="psy")
                    for p in range(NPAIR):
                        nc.tensor.matmul(
                            ps, attnT[j][:, p, qs],
                            woT[:, p, fo * 512:(fo + 1) * 512],
                            start=(p == 0), stop=(p == NPAIR - 1),
                        )
                    ysb = stage.tile([P, 512], f32, tag="ysb")
                    rr_copy(out=ysb, in_=ps)
                    nc.sync.dma_start(
                        out=y[j * QT_TILE + sub * P:
                              j * QT_TILE + (sub + 1) * P,
                              fo * 512:(fo + 1) * 512],
                        in_=ysb)

                # C: V tiles needed by the j=0 attention (keys 0..512*(G0+1))
                tt_first = 4 * (g512[0] + 1)
                for tt in range(tt_first):
                    for fo in range(2):
                        v_step(tt, fo)

                # D: remaining V tiles interleaved with the j=0 attention
                vrest = [(tt, fo) for tt in range(tt_first, T // P)
                         for fo in range(2)]
                asteps = []
                for p in range(NPAIR):
                    asteps += attn_pair_steps(p, 0)
                rate = len(asteps) / max(len(vrest), 1)
                credit, qi = 0.0, 0
                for tt, fo in vrest:
                    v_step(tt, fo)
                    credit += rate
                    while credit >= 1.0 and qi < len(asteps):
                        asteps[qi]()
                        qi += 1
                        credit -= 1.0
                while qi < len(asteps):
                    asteps[qi]()
                    qi += 1

                # E: j=1 attention, j=0 out-proj chunks at pair boundaries
                oj0 = [(0, sub, fo) for sub in range(QT_TILE // P)
                       for fo in range(2)]
                for p in range(NPAIR):
                    for s in attn_pair_steps(p, 1):
                        s()
                    outproj_chunk(*oj0[p])

                # F: j=1 out-proj
                for sub in range(QT_TILE // P):
                    for fo in range(2):
                        outproj_chunk(1, sub, fo)

    nc.compile()
    return nc


def _get_program(parity: int):
    if parity not in _cache:
        _cache[parity] = _build_program(parity)
    return _cache[parity]


def _run_group(nc, in_maps, devices, out_holder, idx):
    """shard_map the program over `devices`, one in_map per device."""
    import jax
    from jax.sharding import Mesh, PartitionSpec
    from jax.experimental.shard_map import shard_map
    import concourse.mybir as mybir
    from concourse.bass2jax import (
        _bass_exec_p, install_neuronx_cc_hook, partition_id_tensor)

    install_neuronx_cc_hook()

    partition_name = (
        nc.partition_id_tensor.name if nc.partition_id_tensor else None)
    in_names, out_names, out_avals, zero_outs = [], [], [], []
    for alloc in nc.m.functions[0].allocations:
        if not isinstance(alloc, mybir.MemoryLocationSet):
            continue
        name = alloc.memorylocations[0].name
        if alloc.kind == "ExternalInput":
            if name != partition_name:
                in_names.append(name)
        elif alloc.kind == "ExternalOutput":
            out_names.append(name)
            shape = tuple(alloc.tensor_shape)
            dtype = mybir.dt.np(alloc.dtype)
            out_avals.append(jax.core.ShapedArray(shape, dtype))
            zero_outs.append(np.zeros(shape, dtype))
    n_params = len(in_names)
    n_outs = len(out_avals)
    all_names = in_names + out_names
    if partition_name is not None:
        all_names.append(partition_name)
    donate = tuple(range(n_params, n_params + n_outs))

    def _body(*args):
        operands = list(args)
        if partition_name is not None:
            operands.append(partition_id_tensor())
        outs = _bass_exec_p.bind(
            *operands,
            out_avals=tuple(out_avals),
            in_names=tuple(all_names),
            out_names=tuple(out_names),
            lowering_input_output_aliases=(),
            sim_require_finite=False,
            sim_require_nnan=False,
            nc=nc,
        )
        return tuple(outs)

    n = len(devices)
    mesh = Mesh(np.asarray(devices), ("core",))
    sharded = jax.jit(
        shard_map(
            _body, mesh=mesh,
            in_specs=(PartitionSpec("core"),) * (n_params + n_outs),
            out_specs=(PartitionSpec("core"),) * n_outs,
            check_rep=False,
        ),
        donate_argnums=donate, keep_unused=True,
    )
    concat_in = [
        np.concatenate([np.asarray(m[name]) for m in in_maps], axis=0)
        for name in in_names
    ]
    concat_zero = [
        np.zeros((n * z.shape[0], *z.shape[1:]), z.dtype) for z in zero_outs
    ]
    out_arrs = sharded(*concat_in, *concat_zero)
    out_holder[idx] = [
        {
            name: np.asarray(out_arrs[i]).reshape(n, *out_avals[i].shape)[c]
            for i, name in enumerate(out_names)
        }
        for c in range(n)
    ]


def kernel(x, attn_mask, w_qkv, w_out):
    """Full inputs in, full output out. attn_mask is all-ones (per the
    problem spec) so masking reduces to the causal structure."""
    import jax

    x = np.asarray(x, dtype=np.float32)
    w_qkv = np.asarray(w_qkv, dtype=np.float32)
    w_out = np.asarray(w_out, dtype=np.float32)

    nc_e = _get_program(0)
    nc_o = _get_program(1)

    devices = jax.devices()
    in_maps = [
        {"x": x[b], "w_qkv": w_qkv, "w_out": w_out} for b in range(B)
    ]

    results = [None, None]
    t_e = threading.Thread(
        target=_run_group, args=(nc_e, in_maps, devices[0:4], results, 0))
    t_o = threading.Thread(
        target=_run_group, args=(nc_o, in_maps, devices[4:8], results, 1))
    t_e.start(); t_o.start()
    t_e.join(); t_o.join()

    y = np.empty((B, T, C), dtype=np.float32)
    for parity, group in enumerate(results):
        for b in range(B):
            y_local = group[b]["y"]          # [TL, C] in local q order
            for j in range(NQT):
                G = QMAP512[parity][j]
                y[b, G * QT_TILE:(G + 1) * QT_TILE, :] = \
                    y_local[j * QT_TILE:(j + 1) * QT_TILE, :]
    return y


# revision 17
# speedup vs baseline: 1.0236x; 1.0236x over previous
"""Causal self-attention on 8 Trainium2 NeuronCores.

Problem: B=4, T=2048, C=1024, H=16, DH=64.
  qkv = x @ w_qkv.T ; causal softmax attention per head ; y = attnout @ w_out.T

Sharding: 8 cores = 4 batches x 2 query-subsets. Each core computes the full
QKV projection for its batch (duplicated within the pair -> no collectives),
then attention for a load-balanced set of query rows (all 16 heads), then
the output projection for its own query rows against the full w_out.

Query balance under causality: global 512-row q-tiles are paired (i, 3-i):
  parity 0 -> q512 tiles [0, 3] (20 key-tiles), parity 1 -> [1, 2] (20).

Everything runs in "transposed space": Q^T/K^T are produced head-pair-stacked
[128=2x64 dh rows, T], scores are computed as S^T (keys on PSUM partitions,
two heads concurrently via PE row-tiling), the softmax denominator is
accumulated on the PE itself (all-ones stationary), PV produces attnout^T
directly (two heads via PE column-tiling), and the output projection consumes
attnout^T as its stationary operand — no transposes in any inner loop.

Schedule: phase-separated with targeted overlap. x^T and the w_v/w_out
transposes stream first, then the K/Q projections (PE-saturated), then just
enough V projection for the first attention q-tile; the remaining V tiles
are interleaved with the j=0 attention, and the j=0 output projection is
interleaved at pair boundaries of the j=1 attention. Diagonal k-tiles slice
scores/exp/PV to the causally-live q-range, the softmax reciprocal uses the
fast approx DVE op (via an SBUF staging copy — the custom DVE op mangles
PSUM reads on HW), and transpose results leave PSUM four blocks per copy.
"""

import threading

import numpy as np

B, T, C = 4, 2048, 1024
H = 16
DH = C // H
P = 128
TL = T // 2          # query rows per core
NPAIR = H // 2       # 8 head-pairs
NCT = C // P         # 8 c-tiles
QT_TILE = 512        # q columns per attention tile
NQT = TL // QT_TILE  # 2 local q-tiles

# local q512-tile -> global q512-tile, per parity (also the Q-proj map)
QMAP512 = [[0, 3], [1, 2]]

_cache = {}


def _build_program(parity: int):
    import concourse.mybir as mybir
    import concourse.tile as tile
    from concourse import bacc
    from concourse.masks import make_identity

    f32 = mybir.dt.float32
    bf16 = mybir.dt.bfloat16

    nc = bacc.Bacc("TRN2", target_bir_lowering=False, debug=False)
    x = nc.dram_tensor("x", [T, C], f32, kind="ExternalInput").ap()
    w_qkv = nc.dram_tensor("w_qkv", [3 * C, C], f32, kind="ExternalInput").ap()
    w_out = nc.dram_tensor("w_out", [C, C], f32, kind="ExternalInput").ap()
    y = nc.dram_tensor("y", [TL, C], f32, kind="ExternalOutput").ap()

    g512 = QMAP512[parity]
    scale = 1.0 / float(np.sqrt(DH))

    with tile.TileContext(nc) as tc:
        with (
            tc.tile_pool(name="res", bufs=1) as res,
            tc.tile_pool(name="stage", bufs=3) as stage,
            tc.tile_pool(name="wtile", bufs=2) as wtile,
            tc.tile_pool(name="work", bufs=2) as work,
            tc.tile_pool(name="rdp", bufs=2) as rdp,
            tc.tile_pool(name="attn", bufs=2) as attnp,
        ):
            ones128 = res.tile([P, P], bf16)
            nc.vector.memset(ones128, 1.0)

            # [128,128] triangle mask: keep (1.0) iff q-col >= key-row
            tri = res.tile([P, P], bf16)
            nc.gpsimd.memset(tri, 1.0)
            nc.gpsimd.affine_select(
                out=tri, in_=tri, compare_op=mybir.AluOpType.is_ge,
                fill=0.0, base=0, pattern=[[1, P]], channel_multiplier=-1,
            )

            ident = res.tile([P, P], bf16)
            make_identity(nc, ident)

            # ---- residents
            kT = res.tile([P, NPAIR, T], bf16)          # K^T   4 MB
            qT = res.tile([P, NPAIR, TL], bf16)         # Q^T   2 MB
            v = res.tile([P, T // P, C], bf16)          # V     4 MB
            wvT = res.tile([P, NCT, C], bf16)           # w_v^T 2 MB
            woT = res.tile([P, NCT, C], bf16)           # w_out^T 2 MB
            xT = res.tile([P, NCT, T], bf16)            # x^T   4 MB

            _cp = [0]

            def rr_copy(out, in_):
                if _cp[0] % 2 == 0:
                    nc.scalar.copy(out=out, in_=in_)
                else:
                    nc.vector.tensor_copy(out=out, in_=in_)
                _cp[0] += 1

            def load_cast(src_ap):
                lf = stage.tile([P, C], f32, tag="ldf")
                nc.sync.dma_start(out=lf, in_=src_ap)
                lb = stage.tile([P, C], bf16, tag="ldb")
                nc.any.tensor_copy(out=lb, in_=lf)
                return lb

            def make_transpose_block(pool):
                def transpose_block(lb, dst, dst_col):
                    for half in range(2):
                        pt4 = pool.tile([P, 4, P], bf16, tag="pt4",
                                        name="pt4")
                        for q in range(4):
                            ct = half * 4 + q
                            nc.tensor.transpose(
                                pt4[:, q, :], lb[:, ct * P:(ct + 1) * P],
                                ident)
                        rr_copy(
                            out=dst[:, half * 4:(half + 1) * 4,
                                    dst_col:dst_col + P],
                            in_=pt4)
                return transpose_block

            # ====== phases A+B: transposes + K/Q projections ======
            with (
                tc.tile_pool(name="pst", bufs=2, space="PSUM") as pst,
                tc.tile_pool(name="psqkv", bufs=4, space="PSUM") as psqkv,
            ):
                transpose_block = make_transpose_block(pst)

                # A: x^T, interleaved with the w_v / w_out block transposes
                for tt in range(T // P):
                    xb = load_cast(x[tt * P:(tt + 1) * P, :])
                    transpose_block(xb, xT, tt * P)
                    if tt < 8:
                        wb = load_cast(w_qkv[(16 + tt) * P:(17 + tt) * P, :])
                        transpose_block(wb, wvT, tt * P)
                    else:
                        fb = tt - 8
                        wb = load_cast(w_out[fb * P:(fb + 1) * P, :])
                        transpose_block(wb, woT, fb * P)

                # B: Q/K projections (Q: only local halves)
                for fb in range(16):                     # 0..7 Q, 8..15 K
                    wb = load_cast(w_qkv[fb * P:(fb + 1) * P, :])
                    wqk = wtile.tile([P, NCT, P], bf16, tag="wqk")
                    transpose_block(wb, wqk, 0)
                    if fb < 8:
                        for u in range(NQT):
                            ps = psqkv.tile([P, 512], f32, tag="psqkv")
                            t0 = g512[u] * 512
                            for ct in range(NCT):
                                nc.tensor.matmul(
                                    ps, wqk[:, ct, :],
                                    xT[:, ct, t0:t0 + 512],
                                    start=(ct == 0), stop=(ct == NCT - 1),
                                )
                            nc.vector.tensor_copy(
                                out=qT[:, fb, u * 512:(u + 1) * 512], in_=ps)
                    else:
                        pr = fb - 8
                        for u in range(4):
                            ps = psqkv.tile([P, 512], f32, tag="psqkv")
                            for ct in range(NCT):
                                nc.tensor.matmul(
                                    ps, wqk[:, ct, :],
                                    xT[:, ct, u * 512:(u + 1) * 512],
                                    start=(ct == 0), stop=(ct == NCT - 1),
                                )
                            nc.vector.tensor_copy(
                                out=kT[:, pr, u * 512:(u + 1) * 512], in_=ps)

            # ====== phases C-F: V projection + attention + out-proj ======
            with (
                tc.tile_pool(name="psv", bufs=2, space="PSUM") as psv,
                tc.tile_pool(name="pss", bufs=2, space="PSUM") as pss,
                tc.tile_pool(name="pso", bufs=2, space="PSUM") as pso,
            ):
                def v_step(tt, fo):
                    ps = psv.tile([P, 512], f32, tag="psv")
                    for ct in range(NCT):
                        nc.tensor.matmul(
                            ps, xT[:, ct, tt * P:(tt + 1) * P],
                            wvT[:, ct, fo * 512:(fo + 1) * 512],
                            start=(ct == 0), stop=(ct == NCT - 1),
                        )
                    nc.vector.tensor_copy(
                        out=v[:, tt, fo * 512:(fo + 1) * 512], in_=ps)

                attnT = [
                    attnp.tile([P, NPAIR, QT_TILE], bf16, tag="attnT",
                               name=f"attnT{j}")
                    for j in range(NQT)
                ]

                def attn_pair_steps(p, j):
                    """List of emission steps for attention of (pair, qtile).
                    Scores for k-tile k are emitted together with the
                    softmax+PV of k-tile k-1 so the PE never queues directly
                    behind the ACT engine's exp."""
                    G = g512[j]
                    nk = 4 * (G + 1)
                    box = {}
                    qA = qT[0:64, p, j * QT_TILE:(j + 1) * QT_TILE]
                    qB = qT[64:128, p, j * QT_TILE:(j + 1) * QT_TILE]

                    def softmax_pv(k, s2, qlo):
                        dj = k - 4 * G
                        p2 = work.tile([P, 2, QT_TILE], bf16, tag="p2")
                        nc.scalar.activation(
                            p2[:, :, qlo:], s2[:, :, qlo:],
                            mybir.ActivationFunctionType.Exp, scale=scale)
                        if dj >= 0:  # triangle block on the diagonal
                            nc.vector.tensor_mul(
                                out=p2[:, :, qlo:qlo + P],
                                in0=p2[:, :, qlo:qlo + P],
                                in1=tri[:, None, :].to_broadcast((P, 2, P)))
                        pA = p2[:, 0, qlo:]
                        pB = p2[:, 1, qlo:]
                        st, sp_ = (k == 0), (k == nk - 1)
                        nc.tensor.matmul(box["pdA"][:, qlo:],
                                         ones128[:, 0:64], pA,
                                         start=st, stop=sp_,
                                         skip_group_check=True)
                        nc.tensor.matmul(box["pdB"][:, qlo:],
                                         ones128[:, 0:64], pB,
                                         start=st, stop=sp_,
                                         skip_group_check=True)
                        nc.tensor.matmul(
                            box["poA"][:, qlo:], v[:, k, p * P:p * P + 64],
                            pA, start=st, stop=sp_, skip_group_check=True)
                        nc.tensor.matmul(
                            box["poB"][:, qlo:],
                            v[:, k, p * P + 64:(p + 1) * P], pB,
                            start=st, stop=sp_, skip_group_check=True)

                    steps = []

                    def mk_ktile(k):
                        dj = k - 4 * G
                        qlo = max(dj, 0) * P

                        def s():
                            if k == 0:
                                poA_t = pso.tile([P, QT_TILE], f32,
                                                 tag="po", name="poA")
                                poB_t = pso.tile([P, QT_TILE], f32,
                                                 tag="po", name="poB")
                                box["poA"] = poA_t[0:64]
                                box["poB"] = poB_t[64:128]
                                box["pdA"] = poB_t[0:64]
                                box["pdB"] = poA_t[64:128]
                            s2 = pss.tile([P, 2, QT_TILE], f32, tag="s2")
                            ks = slice(k * P, (k + 1) * P)
                            nc.tensor.matmul(s2[:, 0, qlo:],
                                             kT[0:64, p, ks], qA[:, qlo:])
                            nc.tensor.matmul(s2[:, 1, qlo:],
                                             kT[64:128, p, ks], qB[:, qlo:])
                            if box.get("pending") is not None:
                                softmax_pv(*box["pending"])
                            box["pending"] = (k, s2, qlo)
                        return s
                    steps += [mk_ktile(k) for k in range(nk)]

                    def s_finish():
                        softmax_pv(*box["pending"])
                        # reciprocal_approx_fast mangles PSUM reads on HW —
                        # stage the denominators through SBUF first.
                        dcp = rdp.tile([P, QT_TILE], f32, tag="dcp")
                        nc.vector.tensor_copy(out=dcp[0:64], in_=box["pdA"])
                        nc.vector.tensor_copy(out=dcp[64:128],
                                              in_=box["pdB"])
                        rD = rdp.tile([P, QT_TILE], f32, tag="rD")
                        nc.vector.reciprocal_approx_fast(rD, dcp)
                        nc.vector.tensor_mul(
                            out=attnT[j][0:64, p, :], in0=box["poA"],
                            in1=rD[0:64])
                        nc.vector.tensor_mul(
                            out=attnT[j][64:128, p, :], in0=box["poB"],
                            in1=rD[64:128])
                    steps.append(s_finish)
                    return steps

                def outproj_chunk(j, sub, fo):
                    qs = slice(sub * P, (sub + 1) * P)
                    ps = pso.tile([P, 512], f32, tag="po", name="psy")
                    for p in range(NPAIR):
                        nc.tensor.matmul(
                            ps, attnT[j][:, p, qs],
                            woT[:, p, fo * 512:(fo + 1) * 512],
                            start=(p == 0), stop=(p == NPAIR - 1),
                        )
                    ysb = stage.tile([P, 512], f32, tag="ysb")
                    rr_copy(out=ysb, in_=ps)
                    nc.sync.dma_start(
                        out=y[j * QT_TILE + sub * P:
                              j * QT_TILE + (sub + 1) * P,
                              fo * 512:(fo + 1) * 512],
                        in_=ysb)

                # C: V tiles needed by the j=0 attention (keys 0..512*(G0+1))
                tt_first = 4 * (g512[0] + 1)
                for tt in range(tt_first):
                    for fo in range(2):
                        v_step(tt, fo)

                # D: remaining V tiles interleaved with the j=0 attention
                vrest = [(tt, fo) for tt in range(tt_first, T // P)
                         for fo in range(2)]
                asteps = []
                for p in range(NPAIR):
                    asteps += attn_pair_steps(p, 0)
                rate = len(asteps) / max(len(vrest), 1)
                credit, qi = 0.0, 0
                for tt, fo in vrest:
                    v_step(tt, fo)
                    credit += rate
                    while credit >= 1.0 and qi < len(asteps):
                        asteps[qi]()
                        qi += 1
                        credit -= 1.0
                while qi < len(asteps):
                    asteps[qi]()
                    qi += 1

                # E: j=1 attention, j=0 out-proj chunks at pair boundaries
                oj0 = [(0, sub, fo) for sub in range(QT_TILE // P)
                       for fo in range(2)]
                for p in range(NPAIR):
                    for s in attn_pair_steps(p, 1):
                        s()
                    outproj_chunk(*oj0[p])

                # F: j=1 out-proj
                for sub in range(QT_TILE // P):
                    for fo in range(2):
                        outproj_chunk(1, sub, fo)

    nc.compile()
    return nc


def _get_program(parity: int):
    if parity not in _cache:
        _cache[parity] = _build_program(parity)
    return _cache[parity]


def _run_group(nc, in_maps, devices, out_holder, idx):
    """shard_map the program over `devices`, one in_map per device."""
    import jax
    from jax.sharding import Mesh, PartitionSpec
    from jax.experimental.shard_map import shard_map
    import concourse.mybir as mybir
    from concourse.bass2jax import (
        _bass_exec_p, install_neuronx_cc_hook, partition_id_tensor)

    install_neuronx_cc_hook()

    partition_name = (
        nc.partition_id_tensor.name if nc.partition_id_tensor else None)
    in_names, out_names, out_avals, zero_outs = [], [], [], []
    for alloc in nc.m.functions[0].allocations:
        if not isinstance(alloc, mybir.MemoryLocationSet):
            continue
        name = alloc.memorylocations[0].name
        if alloc.kind == "ExternalInput":
            if name != partition_name:
                in_names.append(name)
        elif alloc.kind == "ExternalOutput":
            out_names.append(name)
            shape = tuple(alloc.tensor_shape)
            dtype = mybir.dt.np(alloc.dtype)
            out_avals.append(jax.core.ShapedArray(shape, dtype))
            zero_outs.append(np.zeros(shape, dtype))
    n_params = len(in_names)
    n_outs = len(out_avals)
    all_names = in_names + out_names
    if partition_name is not None:
        all_names.append(partition_name)
    donate = tuple(range(n_params, n_params + n_outs))

    def _body(*args):
        operands = list(args)
        if partition_name is not None:
            operands.append(partition_id_tensor())
        outs = _bass_exec_p.bind(
            *operands,
            out_avals=tuple(out_avals),
            in_names=tuple(all_names),
            out_names=tuple(out_names),
            lowering_input_output_aliases=(),
            sim_require_finite=False,
            sim_require_nnan=False,
            nc=nc,
        )
        return tuple(outs)

    n = len(devices)
    mesh = Mesh(np.asarray(devices), ("core",))
    sharded = jax.jit(
        shard_map(
            _body, mesh=mesh,
            in_specs=(PartitionSpec("core"),) * (n_params + n_outs),
            out_specs=(PartitionSpec("core"),) * n_outs,
            check_rep=False,
        ),
        donate_argnums=donate, keep_unused=True,
    )
    concat_in = [
        np.concatenate([np.asarray(m[name]) for m in in_maps], axis=0)
        for name in in_names
    ]
    concat_zero = [
        np.zeros((n * z.shape[0], *z.shape[1:]), z.dtype) for z in zero_outs
    ]
    out_arrs = sharded(*concat_in, *concat_zero)
    out_holder[idx] = [
        {
            name: np.asarray(out_arrs[i]).reshape(n, *out_avals[i].shape)[c]
            for i, name in enumerate(out_names)
        }
        for c in range(n)
    ]


def kernel(x, attn_mask, w_qkv, w_out):
    """Full inputs in, full output out. attn_mask is all-ones (per the
    problem spec) so masking reduces to the causal structure."""
    import jax

    x = np.asarray(x, dtype=np.float32)
    w_qkv = np.asarray(w_qkv, dtype=np.float32)
    w_out = np.asarray(w_out, dtype=np.float32)

    nc_e = _get_program(0)
    nc_o = _get_program(1)

    devices = jax.devices()
    in_maps = [
        {"x": x[b], "w_qkv": w_qkv, "w_out": w_out} for b in range(B)
    ]

    results = [None, None]
    t_e = threading.Thread(
        target=_run_group, args=(nc_e, in_maps, devices[0:4], results, 0))
    t_o = threading.Thread(
        target=_run_group, args=(nc_o, in_maps, devices[4:8], results, 1))
    t_e.start(); t_o.start()
    t_e.join(); t_o.join()

    y = np.empty((B, T, C), dtype=np.float32)
    for parity, group in enumerate(results):
        for b in range(B):
            y_local = group[b]["y"]          # [TL, C] in local q order
            for j in range(NQT):
                G = QMAP512[parity][j]
                y[b, G * QT_TILE:(G + 1) * QT_TILE, :] = \
                    y_local[j * QT_TILE:(j + 1) * QT_TILE, :]
    return y


# revision 19
# speedup vs baseline: 1.0304x; 1.0067x over previous
"""Causal self-attention on 8 Trainium2 NeuronCores.

Problem: B=4, T=2048, C=1024, H=16, DH=64.
  qkv = x @ w_qkv.T ; causal softmax attention per head ; y = attnout @ w_out.T

Sharding: 8 cores = 4 batches x 2 query-subsets. Each core computes the full
QKV projection for its batch (duplicated within the pair -> no collectives),
then attention for a load-balanced set of query rows (all 16 heads), then
the output projection for its own query rows against the full w_out.

Query balance under causality: global 512-row q-tiles are paired (i, 3-i):
  parity 0 -> q512 tiles [0, 3] (20 key-tiles), parity 1 -> [1, 2] (20).

Everything runs in "transposed space": Q^T/K^T are produced head-pair-stacked
[128=2x64 dh rows, T], scores are computed as S^T (keys on PSUM partitions,
two heads concurrently via PE row-tiling), the softmax denominator is
accumulated on the PE itself (all-ones stationary), PV produces attnout^T
directly (two heads via PE column-tiling), and the output projection consumes
attnout^T as its stationary operand — no transposes in any inner loop.

Schedule: phase-separated with targeted overlap. x^T and the w_v/w_out
transposes stream first, then the K/Q projections (PE-saturated), then just
enough V projection for the first attention q-tile; the remaining V tiles
are interleaved with the j=0 attention, and the j=0 output projection is
interleaved at pair boundaries of the j=1 attention. Diagonal k-tiles slice
scores/exp/PV to the causally-live q-range, the softmax reciprocal uses the
fast approx DVE op (via an SBUF staging copy — the custom DVE op mangles
PSUM reads on HW), and transpose results leave PSUM four blocks per copy.
"""

import threading

import numpy as np

B, T, C = 4, 2048, 1024
H = 16
DH = C // H
P = 128
TL = T // 2          # query rows per core
NPAIR = H // 2       # 8 head-pairs
NCT = C // P         # 8 c-tiles
QT_TILE = 512        # q columns per attention tile
NQT = TL // QT_TILE  # 2 local q-tiles

# local q512-tile -> global q512-tile, per parity (also the Q-proj map)
QMAP512 = [[0, 3], [1, 2]]

_cache = {}


def _build_program(parity: int):
    import concourse.mybir as mybir
    import concourse.tile as tile
    from concourse import bacc
    from concourse.masks import make_identity

    f32 = mybir.dt.float32
    bf16 = mybir.dt.bfloat16

    nc = bacc.Bacc("TRN2", target_bir_lowering=False, debug=False)
    x = nc.dram_tensor("x", [T, C], f32, kind="ExternalInput").ap()
    w_qkv = nc.dram_tensor("w_qkv", [3 * C, C], f32, kind="ExternalInput").ap()
    w_out = nc.dram_tensor("w_out", [C, C], f32, kind="ExternalInput").ap()
    y = nc.dram_tensor("y", [TL, C], f32, kind="ExternalOutput").ap()

    g512 = QMAP512[parity]
    scale = 1.0 / float(np.sqrt(DH))

    with tile.TileContext(nc) as tc:
        with (
            tc.tile_pool(name="res", bufs=1) as res,
            tc.tile_pool(name="stage", bufs=3) as stage,
            tc.tile_pool(name="wtile", bufs=2) as wtile,
            tc.tile_pool(name="work", bufs=2) as work,
            tc.tile_pool(name="rdp", bufs=2) as rdp,
            tc.tile_pool(name="attn", bufs=2) as attnp,
        ):
            ones128 = res.tile([P, P], bf16)
            nc.vector.memset(ones128, 1.0)

            # [128,128] triangle mask: keep (1.0) iff q-col >= key-row
            tri = res.tile([P, P], bf16)
            nc.gpsimd.memset(tri, 1.0)
            nc.gpsimd.affine_select(
                out=tri, in_=tri, compare_op=mybir.AluOpType.is_ge,
                fill=0.0, base=0, pattern=[[1, P]], channel_multiplier=-1,
            )

            ident = res.tile([P, P], bf16)
            make_identity(nc, ident)

            # ---- residents
            kT = res.tile([P, NPAIR, T], bf16)          # K^T   4 MB
            qT = res.tile([P, NPAIR, TL], bf16)         # Q^T   2 MB
            v = res.tile([P, T // P, C], bf16)          # V     4 MB
            wvT = res.tile([P, NCT, C], bf16)           # w_v^T 2 MB
            woT = res.tile([P, NCT, C], bf16)           # w_out^T 2 MB
            xT = res.tile([P, NCT, T], bf16)            # x^T   4 MB

            _cp = [0]

            def rr_copy(out, in_):
                if _cp[0] % 2 == 0:
                    nc.scalar.copy(out=out, in_=in_)
                else:
                    nc.vector.tensor_copy(out=out, in_=in_)
                _cp[0] += 1

            def load_cast(src_ap):
                lf = stage.tile([P, C], f32, tag="ldf")
                nc.sync.dma_start(out=lf, in_=src_ap)
                lb = stage.tile([P, C], bf16, tag="ldb")
                nc.any.tensor_copy(out=lb, in_=lf)
                return lb

            def make_transpose_block(pool):
                def transpose_block(lb, dst, dst_col):
                    for half in range(2):
                        pt4 = pool.tile([P, 4, P], bf16, tag="pt4",
                                        name="pt4")
                        for q in range(4):
                            ct = half * 4 + q
                            nc.tensor.transpose(
                                pt4[:, q, :], lb[:, ct * P:(ct + 1) * P],
                                ident)
                        rr_copy(
                            out=dst[:, half * 4:(half + 1) * 4,
                                    dst_col:dst_col + P],
                            in_=pt4)
                return transpose_block

            # ====== phases A+B: transposes + K/Q projections ======
            with (
                tc.tile_pool(name="pst", bufs=4, space="PSUM") as pst,
                tc.tile_pool(name="psqkv", bufs=4, space="PSUM") as psqkv,
            ):
                transpose_block = make_transpose_block(pst)

                # A: x^T, interleaved with the w_v / w_out block transposes
                for tt in range(T // P):
                    xb = load_cast(x[tt * P:(tt + 1) * P, :])
                    transpose_block(xb, xT, tt * P)
                    if tt < 8:
                        wb = load_cast(w_qkv[(16 + tt) * P:(17 + tt) * P, :])
                        transpose_block(wb, wvT, tt * P)
                    else:
                        fb = tt - 8
                        wb = load_cast(w_out[fb * P:(fb + 1) * P, :])
                        transpose_block(wb, woT, fb * P)

                # B: Q/K projections (Q: only local halves)
                for fb in range(16):                     # 0..7 Q, 8..15 K
                    wb = load_cast(w_qkv[fb * P:(fb + 1) * P, :])
                    wqk = wtile.tile([P, NCT, P], bf16, tag="wqk")
                    transpose_block(wb, wqk, 0)
                    if fb < 8:
                        for u in range(NQT):
                            ps = psqkv.tile([P, 512], f32, tag="psqkv")
                            t0 = g512[u] * 512
                            for ct in range(NCT):
                                nc.tensor.matmul(
                                    ps, wqk[:, ct, :],
                                    xT[:, ct, t0:t0 + 512],
                                    start=(ct == 0), stop=(ct == NCT - 1),
                                )
                            nc.vector.tensor_copy(
                                out=qT[:, fb, u * 512:(u + 1) * 512], in_=ps)
                    else:
                        pr = fb - 8
                        for u in range(4):
                            ps = psqkv.tile([P, 512], f32, tag="psqkv")
                            for ct in range(NCT):
                                nc.tensor.matmul(
                                    ps, wqk[:, ct, :],
                                    xT[:, ct, u * 512:(u + 1) * 512],
                                    start=(ct == 0), stop=(ct == NCT - 1),
                                )
                            nc.vector.tensor_copy(
                                out=kT[:, pr, u * 512:(u + 1) * 512], in_=ps)

            # ====== phases C-F: V projection + attention + out-proj ======
            with (
                tc.tile_pool(name="psv", bufs=2, space="PSUM") as psv,
                tc.tile_pool(name="pss", bufs=2, space="PSUM") as pss,
                tc.tile_pool(name="pso", bufs=2, space="PSUM") as pso,
            ):
                def v_step(tt, fo):
                    ps = psv.tile([P, 512], f32, tag="psv")
                    for ct in range(NCT):
                        nc.tensor.matmul(
                            ps, xT[:, ct, tt * P:(tt + 1) * P],
                            wvT[:, ct, fo * 512:(fo + 1) * 512],
                            start=(ct == 0), stop=(ct == NCT - 1),
                        )
                    nc.vector.tensor_copy(
                        out=v[:, tt, fo * 512:(fo + 1) * 512], in_=ps)

                attnT = [
                    attnp.tile([P, NPAIR, QT_TILE], bf16, tag="attnT",
                               name=f"attnT{j}")
                    for j in range(NQT)
                ]

                def attn_pair_steps(p, j):
                    """List of emission steps for attention of (pair, qtile).
                    Scores for k-tile k are emitted together with the
                    softmax+PV of k-tile k-1 so the PE never queues directly
                    behind the ACT engine's exp."""
                    G = g512[j]
                    nk = 4 * (G + 1)
                    box = {}
                    qA = qT[0:64, p, j * QT_TILE:(j + 1) * QT_TILE]
                    qB = qT[64:128, p, j * QT_TILE:(j + 1) * QT_TILE]

                    def softmax_pv(k, s2, qlo):
                        dj = k - 4 * G
                        p2 = work.tile([P, 2, QT_TILE], bf16, tag="p2")
                        nc.scalar.activation(
                            p2[:, :, qlo:], s2[:, :, qlo:],
                            mybir.ActivationFunctionType.Exp, scale=scale)
                        if dj >= 0:  # triangle block on the diagonal
                            nc.vector.tensor_mul(
                                out=p2[:, :, qlo:qlo + P],
                                in0=p2[:, :, qlo:qlo + P],
                                in1=tri[:, None, :].to_broadcast((P, 2, P)))
                        pA = p2[:, 0, qlo:]
                        pB = p2[:, 1, qlo:]
                        st, sp_ = (k == 0), (k == nk - 1)
                        nc.tensor.matmul(box["pdA"][:, qlo:],
                                         ones128[:, 0:64], pA,
                                         start=st, stop=sp_,
                                         skip_group_check=True)
                        nc.tensor.matmul(box["pdB"][:, qlo:],
                                         ones128[:, 0:64], pB,
                                         start=st, stop=sp_,
                                         skip_group_check=True)
                        nc.tensor.matmul(
                            box["poA"][:, qlo:], v[:, k, p * P:p * P + 64],
                            pA, start=st, stop=sp_, skip_group_check=True)
                        nc.tensor.matmul(
                            box["poB"][:, qlo:],
                            v[:, k, p * P + 64:(p + 1) * P], pB,
                            start=st, stop=sp_, skip_group_check=True)

                    steps = []

                    def mk_ktile(k):
                        dj = k - 4 * G
                        qlo = max(dj, 0) * P

                        def s():
                            if k == 0:
                                poA_t = pso.tile([P, QT_TILE], f32,
                                                 tag="po", name="poA")
                                poB_t = pso.tile([P, QT_TILE], f32,
                                                 tag="po", name="poB")
                                box["poA"] = poA_t[0:64]
                                box["poB"] = poB_t[64:128]
                                box["pdA"] = poB_t[0:64]
                                box["pdB"] = poA_t[64:128]
                            s2 = pss.tile([P, 2, QT_TILE], f32, tag="s2")
                            ks = slice(k * P, (k + 1) * P)
                            nc.tensor.matmul(s2[:, 0, qlo:],
                                             kT[0:64, p, ks], qA[:, qlo:])
                            nc.tensor.matmul(s2[:, 1, qlo:],
                                             kT[64:128, p, ks], qB[:, qlo:])
                            if box.get("pending") is not None:
                                softmax_pv(*box["pending"])
                            box["pending"] = (k, s2, qlo)
                        return s
                    steps += [mk_ktile(k) for k in range(nk)]

                    def s_finish():
                        softmax_pv(*box["pending"])
                        # reciprocal_approx_fast mangles PSUM reads on HW —
                        # stage the denominators through SBUF first.
                        dcp = rdp.tile([P, QT_TILE], f32, tag="dcp")
                        nc.vector.tensor_copy(out=dcp[0:64], in_=box["pdA"])
                        nc.vector.tensor_copy(out=dcp[64:128],
                                              in_=box["pdB"])
                        rD = rdp.tile([P, QT_TILE], f32, tag="rD")
                        nc.vector.reciprocal_approx_fast(rD, dcp)
                        nc.vector.tensor_mul(
                            out=attnT[j][0:64, p, :], in0=box["poA"],
                            in1=rD[0:64])
                        nc.vector.tensor_mul(
                            out=attnT[j][64:128, p, :], in0=box["poB"],
                            in1=rD[64:128])
                    steps.append(s_finish)
                    return steps

                def outproj_chunk(j, sub, fo):
                    qs = slice(sub * P, (sub + 1) * P)
                    ps = pso.tile([P, 512], f32, tag="po", name="psy")
                    for p in range(NPAIR):
                        nc.tensor.matmul(
                            ps, attnT[j][:, p, qs],
                            woT[:, p, fo * 512:(fo + 1) * 512],
                            start=(p == 0), stop=(p == NPAIR - 1),
                        )
                    ysb = stage.tile([P, 512], f32, tag="ysb")
                    # Vector-pinned: Scalar is the exp pacer during j=1
                    nc.vector.tensor_copy(out=ysb, in_=ps)
                    nc.sync.dma_start(
                        out=y[j * QT_TILE + sub * P:
                              j * QT_TILE + (sub + 1) * P,
                              fo * 512:(fo + 1) * 512],
                        in_=ysb)

                # C: V tiles needed by the j=0 attention (keys 0..512*(G0+1))
                tt_first = 4 * (g512[0] + 1)
                for tt in range(tt_first):
                    for fo in range(2):
                        v_step(tt, fo)

                # D: remaining V tiles interleaved with the j=0 attention
                vrest = [(tt, fo) for tt in range(tt_first, T // P)
                         for fo in range(2)]
                asteps = []
                for p in range(NPAIR):
                    asteps += attn_pair_steps(p, 0)
                rate = len(asteps) / max(len(vrest), 1)
                credit, qi = 0.0, 0
                for tt, fo in vrest:
                    v_step(tt, fo)
                    credit += rate
                    while credit >= 1.0 and qi < len(asteps):
                        asteps[qi]()
                        qi += 1
                        credit -= 1.0
                while qi < len(asteps):
                    asteps[qi]()
                    qi += 1

                # E: j=1 attention, j=0 out-proj chunks at pair boundaries
                oj0 = [(0, sub, fo) for sub in range(QT_TILE // P)
                       for fo in range(2)]
                for p in range(NPAIR):
                    for s in attn_pair_steps(p, 1):
                        s()
                    outproj_chunk(*oj0[p])

                # F: j=1 out-proj
                for sub in range(QT_TILE // P):
                    for fo in range(2):
                        outproj_chunk(1, sub, fo)

    nc.compile()
    return nc


def _get_program(parity: int):
    if parity not in _cache:
        _cache[parity] = _build_program(parity)
    return _cache[parity]


def _run_group(nc, in_maps, devices, out_holder, idx):
    """shard_map the program over `devices`, one in_map per device."""
    import jax
    from jax.sharding import Mesh, PartitionSpec
    from jax.experimental.shard_map import shard_map
    import concourse.mybir as mybir
    from concourse.bass2jax import (
        _bass_exec_p, install_neuronx_cc_hook, partition_id_tensor)

    install_neuronx_cc_hook()

    partition_name = (
        nc.partition_id_tensor.name if nc.partition_id_tensor else None)
    in_names, out_names, out_avals, zero_outs = [], [], [], []
    for alloc in nc.m.functions[0].allocations:
        if not isinstance(alloc, mybir.MemoryLocationSet):
            continue
        name = alloc.memorylocations[0].name
        if alloc.kind == "ExternalInput":
            if name != partition_name:
                in_names.append(name)
        elif alloc.kind == "ExternalOutput":
            out_names.append(name)
            shape = tuple(alloc.tensor_shape)
            dtype = mybir.dt.np(alloc.dtype)
            out_avals.append(jax.core.ShapedArray(shape, dtype))
            zero_outs.append(np.zeros(shape, dtype))
    n_params = len(in_names)
    n_outs = len(out_avals)
    all_names = in_names + out_names
    if partition_name is not None:
        all_names.append(partition_name)
    donate = tuple(range(n_params, n_params + n_outs))

    def _body(*args):
        operands = list(args)
        if partition_name is not None:
            operands.append(partition_id_tensor())
        outs = _bass_exec_p.bind(
            *operands,
            out_avals=tuple(out_avals),
            in_names=tuple(all_names),
            out_names=tuple(out_names),
            lowering_input_output_aliases=(),
            sim_require_finite=False,
            sim_require_nnan=False,
            nc=nc,
        )
        return tuple(outs)

    n = len(devices)
    mesh = Mesh(np.asarray(devices), ("core",))
    sharded = jax.jit(
        shard_map(
            _body, mesh=mesh,
            in_specs=(PartitionSpec("core"),) * (n_params + n_outs),
            out_specs=(PartitionSpec("core"),) * n_outs,
            check_rep=False,
        ),
        donate_argnums=donate, keep_unused=True,
    )
    concat_in = [
        np.concatenate([np.asarray(m[name]) for m in in_maps], axis=0)
        for name in in_names
    ]
    concat_zero = [
        np.zeros((n * z.shape[0], *z.shape[1:]), z.dtype) for z in zero_outs
    ]
    out_arrs = sharded(*concat_in, *concat_zero)
    out_holder[idx] = [
        {
            name: np.asarray(out_arrs[i]).reshape(n, *out_avals[i].shape)[c]
            for i, name in enumerate(out_names)
        }
        for c in range(n)
    ]


def kernel(x, attn_mask, w_qkv, w_out):
    """Full inputs in, full output out. attn_mask is all-ones (per the
    problem spec) so masking reduces to the causal structure."""
    import jax

    x = np.asarray(x, dtype=np.float32)
    w_qkv = np.asarray(w_qkv, dtype=np.float32)
    w_out = np.asarray(w_out, dtype=np.float32)

    nc_e = _get_program(0)
    nc_o = _get_program(1)

    devices = jax.devices()
    in_maps = [
        {"x": x[b], "w_qkv": w_qkv, "w_out": w_out} for b in range(B)
    ]

    results = [None, None]
    t_e = threading.Thread(
        target=_run_group, args=(nc_e, in_maps, devices[0:4], results, 0))
    t_o = threading.Thread(
        target=_run_group, args=(nc_o, in_maps, devices[4:8], results, 1))
    t_e.start(); t_o.start()
    t_e.join(); t_o.join()

    y = np.empty((B, T, C), dtype=np.float32)
    for parity, group in enumerate(results):
        for b in range(B):
            y_local = group[b]["y"]          # [TL, C] in local q order
            for j in range(NQT):
                G = QMAP512[parity][j]
                y[b, G * QT_TILE:(G + 1) * QT_TILE, :] = \
                    y_local[j * QT_TILE:(j + 1) * QT_TILE, :]
    return y


# revision 20
# speedup vs baseline: 1.0467x; 1.0158x over previous
"""Causal self-attention on 8 Trainium2 NeuronCores.

Problem: B=4, T=2048, C=1024, H=16, DH=64.
  qkv = x @ w_qkv.T ; causal softmax attention per head ; y = attnout @ w_out.T

Sharding: 8 cores = 4 batches x 2 query-subsets. Each core computes the full
QKV projection for its batch (duplicated within the pair -> no collectives),
then attention for a load-balanced set of query rows (all 16 heads), then
the output projection for its own query rows against the full w_out.

Query balance under causality: global 512-row q-tiles are paired (i, 3-i):
  parity 0 -> q512 tiles [0, 3] (20 key-tiles), parity 1 -> [1, 2] (20).

Everything runs in "transposed space": Q^T/K^T are produced head-pair-stacked
[128=2x64 dh rows, T], scores are computed as S^T (keys on PSUM partitions,
two heads concurrently via PE row-tiling), the softmax denominator is
accumulated on the PE itself (all-ones stationary), PV produces attnout^T
directly (two heads via PE column-tiling), and the output projection consumes
attnout^T as its stationary operand — no transposes in any inner loop.

Schedule: phase-separated with targeted overlap. x^T and the w_v/w_out
transposes stream first, then the K/Q projections (PE-saturated), then just
enough V projection for the first attention q-tile; the remaining V tiles
are interleaved with the j=0 attention, and the j=0 output projection is
interleaved at pair boundaries of the j=1 attention. Diagonal k-tiles slice
scores/exp/PV to the causally-live q-range, the softmax reciprocal uses the
fast approx DVE op (via an SBUF staging copy — the custom DVE op mangles
PSUM reads on HW), and transpose results leave PSUM four blocks per copy.
"""

import threading

import numpy as np

B, T, C = 4, 2048, 1024
H = 16
DH = C // H
P = 128
TL = T // 2          # query rows per core
NPAIR = H // 2       # 8 head-pairs
NCT = C // P         # 8 c-tiles
QT_TILE = 512        # q columns per attention tile
NQT = TL // QT_TILE  # 2 local q-tiles

# local q512-tile -> global q512-tile, per parity (also the Q-proj map)
QMAP512 = [[0, 3], [1, 2]]

_cache = {}


def _build_program(parity: int):
    import concourse.mybir as mybir
    import concourse.tile as tile
    from concourse import bacc
    from concourse.masks import make_identity

    f32 = mybir.dt.float32
    bf16 = mybir.dt.bfloat16

    nc = bacc.Bacc("TRN2", target_bir_lowering=False, debug=False)
    x = nc.dram_tensor("x", [T, C], f32, kind="ExternalInput").ap()
    w_qkv = nc.dram_tensor("w_qkv", [3 * C, C], f32, kind="ExternalInput").ap()
    w_out = nc.dram_tensor("w_out", [C, C], f32, kind="ExternalInput").ap()
    y = nc.dram_tensor("y", [TL, C], f32, kind="ExternalOutput").ap()

    g512 = QMAP512[parity]
    scale = 1.0 / float(np.sqrt(DH))

    with tile.TileContext(nc) as tc:
        with (
            tc.tile_pool(name="res", bufs=1) as res,
            tc.tile_pool(name="stage", bufs=3) as stage,
            tc.tile_pool(name="wtile", bufs=2) as wtile,
            tc.tile_pool(name="work", bufs=2) as work,
            tc.tile_pool(name="rdp", bufs=2) as rdp,
            tc.tile_pool(name="attn", bufs=2) as attnp,
        ):
            ones128 = res.tile([P, P], bf16)
            nc.vector.memset(ones128, 1.0)

            # [128,128] triangle mask: keep (1.0) iff q-col >= key-row
            tri = res.tile([P, P], bf16)
            nc.gpsimd.memset(tri, 1.0)
            nc.gpsimd.affine_select(
                out=tri, in_=tri, compare_op=mybir.AluOpType.is_ge,
                fill=0.0, base=0, pattern=[[1, P]], channel_multiplier=-1,
            )

            ident = res.tile([P, P], bf16)
            make_identity(nc, ident)

            # ---- residents
            kT = res.tile([P, NPAIR, T], bf16)          # K^T   4 MB
            qT = res.tile([P, NPAIR, TL], bf16)         # Q^T   2 MB
            v = res.tile([P, T // P, C], bf16)          # V     4 MB
            wvT = res.tile([P, NCT, C], bf16)           # w_v^T 2 MB
            woT = res.tile([P, NCT, C], bf16)           # w_out^T 2 MB
            xT = res.tile([P, NCT, T], bf16)            # x^T   4 MB

            _cp = [0]

            def rr_copy(out, in_):
                if _cp[0] % 2 == 0:
                    nc.scalar.copy(out=out, in_=in_)
                else:
                    nc.vector.tensor_copy(out=out, in_=in_)
                _cp[0] += 1

            def load_cast(src_ap):
                lf = stage.tile([P, C], f32, tag="ldf")
                nc.sync.dma_start(out=lf, in_=src_ap)
                lb = stage.tile([P, C], bf16, tag="ldb")
                nc.any.tensor_copy(out=lb, in_=lf)
                return lb

            def make_transpose_block(pool):
                def transpose_block(lb, dst, dst_col):
                    for half in range(2):
                        pt4 = pool.tile([P, 4, P], bf16, tag="pt4",
                                        name="pt4")
                        for q in range(4):
                            ct = half * 4 + q
                            nc.tensor.transpose(
                                pt4[:, q, :], lb[:, ct * P:(ct + 1) * P],
                                ident)
                        rr_copy(
                            out=dst[:, half * 4:(half + 1) * 4,
                                    dst_col:dst_col + P],
                            in_=pt4)
                return transpose_block

            # ====== phases A+B: transposes + K/Q projections ======
            with (
                tc.tile_pool(name="pst", bufs=4, space="PSUM") as pst,
                tc.tile_pool(name="psqkv", bufs=4, space="PSUM") as psqkv,
            ):
                transpose_block = make_transpose_block(pst)

                # A: x^T, interleaved with the w_v / w_out block transposes
                for tt in range(T // P):
                    xb = load_cast(x[tt * P:(tt + 1) * P, :])
                    transpose_block(xb, xT, tt * P)
                    if tt < 8:
                        wb = load_cast(w_qkv[(16 + tt) * P:(17 + tt) * P, :])
                        transpose_block(wb, wvT, tt * P)
                    else:
                        fb = tt - 8
                        wb = load_cast(w_out[fb * P:(fb + 1) * P, :])
                        transpose_block(wb, woT, fb * P)

                # B: Q/K projections (Q: only local halves)
                for fb in range(16):                     # 0..7 Q, 8..15 K
                    wb = load_cast(w_qkv[fb * P:(fb + 1) * P, :])
                    wqk = wtile.tile([P, NCT, P], bf16, tag="wqk")
                    transpose_block(wb, wqk, 0)
                    if fb < 8:
                        for u in range(NQT):
                            ps = psqkv.tile([P, 512], f32, tag="psqkv")
                            t0 = g512[u] * 512
                            for ct in range(NCT):
                                nc.tensor.matmul(
                                    ps, wqk[:, ct, :],
                                    xT[:, ct, t0:t0 + 512],
                                    start=(ct == 0), stop=(ct == NCT - 1),
                                )
                            nc.vector.tensor_copy(
                                out=qT[:, fb, u * 512:(u + 1) * 512], in_=ps)
                    else:
                        pr = fb - 8
                        for u in range(4):
                            ps = psqkv.tile([P, 512], f32, tag="psqkv")
                            for ct in range(NCT):
                                nc.tensor.matmul(
                                    ps, wqk[:, ct, :],
                                    xT[:, ct, u * 512:(u + 1) * 512],
                                    start=(ct == 0), stop=(ct == NCT - 1),
                                )
                            nc.vector.tensor_copy(
                                out=kT[:, pr, u * 512:(u + 1) * 512], in_=ps)

            # ====== phases C-F: V projection + attention + out-proj ======
            with (
                tc.tile_pool(name="psv", bufs=2, space="PSUM") as psv,
                tc.tile_pool(name="pss", bufs=2, space="PSUM") as pss,
                tc.tile_pool(name="pso", bufs=2, space="PSUM") as pso,
            ):
                def v_step(tt, fo):
                    ps = psv.tile([P, 512], f32, tag="psv")
                    for ct in range(NCT):
                        nc.tensor.matmul(
                            ps, xT[:, ct, tt * P:(tt + 1) * P],
                            wvT[:, ct, fo * 512:(fo + 1) * 512],
                            start=(ct == 0), stop=(ct == NCT - 1),
                        )
                    nc.vector.tensor_copy(
                        out=v[:, tt, fo * 512:(fo + 1) * 512], in_=ps)

                attnT = [
                    attnp.tile([P, NPAIR, QT_TILE], bf16, tag="attnT",
                               name=f"attnT{j}")
                    for j in range(NQT)
                ]

                def attn_pair_steps(p, j):
                    """List of emission steps for attention of (pair, qtile).
                    Scores for k-tile k are emitted together with the
                    softmax+PV of k-tile k-1 so the PE never queues directly
                    behind the ACT engine's exp."""
                    G = g512[j]
                    nk = 4 * (G + 1)
                    box = {}
                    qA = qT[0:64, p, j * QT_TILE:(j + 1) * QT_TILE]
                    qB = qT[64:128, p, j * QT_TILE:(j + 1) * QT_TILE]

                    def softmax_pv(k, s2, qlo):
                        dj = k - 4 * G
                        p2 = work.tile([P, 2, QT_TILE], bf16, tag="p2")
                        nc.scalar.activation(
                            p2[:, :, qlo:], s2[:, :, qlo:],
                            mybir.ActivationFunctionType.Exp, scale=scale)
                        if dj >= 0:  # triangle block on the diagonal
                            nc.vector.tensor_mul(
                                out=p2[:, :, qlo:qlo + P],
                                in0=p2[:, :, qlo:qlo + P],
                                in1=tri[:, None, :].to_broadcast((P, 2, P)))
                        pA = p2[:, 0, qlo:]
                        pB = p2[:, 1, qlo:]
                        st, sp_ = (k == 0), (k == nk - 1)
                        nc.tensor.matmul(box["pdA"][:, qlo:],
                                         ones128[:, 0:64], pA,
                                         start=st, stop=sp_,
                                         skip_group_check=True)
                        nc.tensor.matmul(box["pdB"][:, qlo:],
                                         ones128[:, 0:64], pB,
                                         start=st, stop=sp_,
                                         skip_group_check=True)
                        nc.tensor.matmul(
                            box["poA"][:, qlo:], v[:, k, p * P:p * P + 64],
                            pA, start=st, stop=sp_, skip_group_check=True)
                        nc.tensor.matmul(
                            box["poB"][:, qlo:],
                            v[:, k, p * P + 64:(p + 1) * P], pB,
                            start=st, stop=sp_, skip_group_check=True)

                    steps = []

                    def mk_ktile(k):
                        dj = k - 4 * G
                        qlo = max(dj, 0) * P

                        def s():
                            if k == 0:
                                poA_t = pso.tile([P, QT_TILE], f32,
                                                 tag="po", name="poA")
                                poB_t = pso.tile([P, QT_TILE], f32,
                                                 tag="po", name="poB")
                                box["poA"] = poA_t[0:64]
                                box["poB"] = poB_t[64:128]
                                box["pdA"] = poB_t[0:64]
                                box["pdB"] = poA_t[64:128]
                            s2 = pss.tile([P, 2, QT_TILE], f32, tag="s2")
                            ks = slice(k * P, (k + 1) * P)
                            nc.tensor.matmul(s2[:, 0, qlo:],
                                             kT[0:64, p, ks], qA[:, qlo:])
                            nc.tensor.matmul(s2[:, 1, qlo:],
                                             kT[64:128, p, ks], qB[:, qlo:])
                            if box.get("pending") is not None:
                                softmax_pv(*box["pending"])
                            box["pending"] = (k, s2, qlo)
                        return s
                    steps += [mk_ktile(k) for k in range(nk)]

                    def s_finish():
                        softmax_pv(*box["pending"])
                        # reciprocal_approx_fast mangles PSUM reads on HW —
                        # stage the denominators through SBUF first.
                        dcp = rdp.tile([P, QT_TILE], f32, tag="dcp")
                        nc.vector.tensor_copy(out=dcp[0:64], in_=box["pdA"])
                        nc.vector.tensor_copy(out=dcp[64:128],
                                              in_=box["pdB"])
                        rD = rdp.tile([P, QT_TILE], f32, tag="rD")
                        nc.vector.reciprocal_approx_fast(rD, dcp)
                        nc.vector.tensor_mul(
                            out=attnT[j][0:64, p, :], in0=box["poA"],
                            in1=rD[0:64])
                        nc.vector.tensor_mul(
                            out=attnT[j][64:128, p, :], in0=box["poB"],
                            in1=rD[64:128])
                    steps.append(s_finish)
                    return steps

                def outproj_chunk(j, sub, fo):
                    qs = slice(sub * P, (sub + 1) * P)
                    ps = pso.tile([P, 512], f32, tag="po", name="psy")
                    for p in range(NPAIR):
                        nc.tensor.matmul(
                            ps, attnT[j][:, p, qs],
                            woT[:, p, fo * 512:(fo + 1) * 512],
                            start=(p == 0), stop=(p == NPAIR - 1),
                        )
                    ysb = stage.tile([P, 512], f32, tag="ysb")
                    # Vector-pinned: Scalar is the exp pacer during j=1
                    nc.vector.tensor_copy(out=ysb, in_=ps)
                    nc.sync.dma_start(
                        out=y[j * QT_TILE + sub * P:
                              j * QT_TILE + (sub + 1) * P,
                              fo * 512:(fo + 1) * 512],
                        in_=ysb)

                # C: V tiles needed by the j=0 attention (keys 0..512*(G0+1))
                tt_first = 4 * (g512[0] + 1)
                for tt in range(tt_first):
                    for fo in range(2):
                        v_step(tt, fo)

                # D: remaining V tiles interleaved with the j=0 attention
                vrest = [(tt, fo) for tt in range(tt_first, T // P)
                         for fo in range(2)]
                asteps = []
                for p in range(NPAIR):
                    asteps += attn_pair_steps(p, 0)
                rate = len(asteps) / max(len(vrest), 1)
                credit, qi = 0.0, 0
                for tt, fo in vrest:
                    v_step(tt, fo)
                    credit += rate
                    while credit >= 1.0 and qi < len(asteps):
                        asteps[qi]()
                        qi += 1
                        credit -= 1.0
                while qi < len(asteps):
                    asteps[qi]()
                    qi += 1

                # E: j=1 attention, cross-pair software-pipelined: pair p's
                # finish + the j=0 out-proj chunk are emitted after pair
                # p+1's first scores, so the ACT exp stream never drains at
                # a pair boundary.
                oj0 = [(0, sub, fo) for sub in range(QT_TILE // P)
                       for fo in range(2)]
                prev_tail = []
                for p in range(NPAIR):
                    ks = attn_pair_steps(p, 1)
                    ks[0]()
                    for s in prev_tail:
                        s()
                    for s in ks[1:-1]:
                        s()
                    prev_tail = [ks[-1],
                                 (lambda p=p: outproj_chunk(*oj0[p]))]
                for s in prev_tail:
                    s()

                # F: j=1 out-proj
                for sub in range(QT_TILE // P):
                    for fo in range(2):
                        outproj_chunk(1, sub, fo)

    nc.compile()
    return nc


def _get_program(parity: int):
    if parity not in _cache:
        _cache[parity] = _build_program(parity)
    return _cache[parity]


def _run_group(nc, in_maps, devices, out_holder, idx):
    """shard_map the program over `devices`, one in_map per device."""
    import jax
    from jax.sharding import Mesh, PartitionSpec
    from jax.experimental.shard_map import shard_map
    import concourse.mybir as mybir
    from concourse.bass2jax import (
        _bass_exec_p, install_neuronx_cc_hook, partition_id_tensor)

    install_neuronx_cc_hook()

    partition_name = (
        nc.partition_id_tensor.name if nc.partition_id_tensor else None)
    in_names, out_names, out_avals, zero_outs = [], [], [], []
    for alloc in nc.m.functions[0].allocations:
        if not isinstance(alloc, mybir.MemoryLocationSet):
            continue
        name = alloc.memorylocations[0].name
        if alloc.kind == "ExternalInput":
            if name != partition_name:
                in_names.append(name)
        elif alloc.kind == "ExternalOutput":
            out_names.append(name)
            shape = tuple(alloc.tensor_shape)
            dtype = mybir.dt.np(alloc.dtype)
            out_avals.append(jax.core.ShapedArray(shape, dtype))
            zero_outs.append(np.zeros(shape, dtype))
    n_params = len(in_names)
    n_outs = len(out_avals)
    all_names = in_names + out_names
    if partition_name is not None:
        all_names.append(partition_name)
    donate = tuple(range(n_params, n_params + n_outs))

    def _body(*args):
        operands = list(args)
        if partition_name is not None:
            operands.append(partition_id_tensor())
        outs = _bass_exec_p.bind(
            *operands,
            out_avals=tuple(out_avals),
            in_names=tuple(all_names),
            out_names=tuple(out_names),
            lowering_input_output_aliases=(),
            sim_require_finite=False,
            sim_require_nnan=False,
            nc=nc,
        )
        return tuple(outs)

    n = len(devices)
    mesh = Mesh(np.asarray(devices), ("core",))
    sharded = jax.jit(
        shard_map(
            _body, mesh=mesh,
            in_specs=(PartitionSpec("core"),) * (n_params + n_outs),
            out_specs=(PartitionSpec("core"),) * n_outs,
            check_rep=False,
        ),
        donate_argnums=donate, keep_unused=True,
    )
    concat_in = [
        np.concatenate([np.asarray(m[name]) for m in in_maps], axis=0)
        for name in in_names
    ]
    concat_zero = [
        np.zeros((n * z.shape[0], *z.shape[1:]), z.dtype) for z in zero_outs
    ]
    out_arrs = sharded(*concat_in, *concat_zero)
    out_holder[idx] = [
        {
            name: np.asarray(out_arrs[i]).reshape(n, *out_avals[i].shape)[c]
            for i, name in enumerate(out_names)
        }
        for c in range(n)
    ]


def kernel(x, attn_mask, w_qkv, w_out):
    """Full inputs in, full output out. attn_mask is all-ones (per the
    problem spec) so masking reduces to the causal structure."""
    import jax

    x = np.asarray(x, dtype=np.float32)
    w_qkv = np.asarray(w_qkv, dtype=np.float32)
    w_out = np.asarray(w_out, dtype=np.float32)

    nc_e = _get_program(0)
    nc_o = _get_program(1)

    devices = jax.devices()
    in_maps = [
        {"x": x[b], "w_qkv": w_qkv, "w_out": w_out} for b in range(B)
    ]

    results = [None, None]
    t_e = threading.Thread(
        target=_run_group, args=(nc_e, in_maps, devices[0:4], results, 0))
    t_o = threading.Thread(
        target=_run_group, args=(nc_o, in_maps, devices[4:8], results, 1))
    t_e.start(); t_o.start()
    t_e.join(); t_o.join()

    y = np.empty((B, T, C), dtype=np.float32)
    for parity, group in enumerate(results):
        for b in range(B):
            y_local = group[b]["y"]          # [TL, C] in local q order
            for j in range(NQT):
                G = QMAP512[parity][j]
                y[b, G * QT_TILE:(G + 1) * QT_TILE, :] = \
                    y_local[j * QT_TILE:(j + 1) * QT_TILE, :]
    return y


# revision 21
# speedup vs baseline: 1.0747x; 1.0268x over previous
"""Causal self-attention on 8 Trainium2 NeuronCores.

Problem: B=4, T=2048, C=1024, H=16, DH=64.
  qkv = x @ w_qkv.T ; causal softmax attention per head ; y = attnout @ w_out.T

Sharding: 8 cores = 4 batches x 2 query-subsets. Each core computes the full
QKV projection for its batch (duplicated within the pair -> no collectives),
then attention for a load-balanced set of query rows (all 16 heads), then
the output projection for its own query rows against the full w_out.

Query balance under causality: global 512-row q-tiles are paired (i, 3-i):
  parity 0 -> q512 tiles [0, 3] (20 key-tiles), parity 1 -> [1, 2] (20).

Everything runs in "transposed space": Q^T/K^T are produced head-pair-stacked
[128=2x64 dh rows, T], scores are computed as S^T (keys on PSUM partitions,
two heads concurrently via PE row-tiling), the softmax denominator is
accumulated on the PE itself (all-ones stationary), PV produces attnout^T
directly (two heads via PE column-tiling), and the output projection consumes
attnout^T as its stationary operand — no transposes in any inner loop.

Schedule: phase-separated with targeted overlap. x^T and the w_v/w_out
transposes stream first, then the K/Q projections (PE-saturated), then just
enough V projection for the first attention q-tile; the remaining V tiles
are interleaved with the j=0 attention, and the j=0 output projection is
interleaved at pair boundaries of the j=1 attention. Diagonal k-tiles slice
scores/exp/PV to the causally-live q-range, the softmax reciprocal uses the
fast approx DVE op (via an SBUF staging copy — the custom DVE op mangles
PSUM reads on HW), and transpose results leave PSUM four blocks per copy.
"""

import threading

import numpy as np

B, T, C = 4, 2048, 1024
H = 16
DH = C // H
P = 128
TL = T // 2          # query rows per core
NPAIR = H // 2       # 8 head-pairs
NCT = C // P         # 8 c-tiles
QT_TILE = 512        # q columns per attention tile
NQT = TL // QT_TILE  # 2 local q-tiles

# local q512-tile -> global q512-tile, per parity (also the Q-proj map)
QMAP512 = [[0, 3], [1, 2]]

_cache = {}


def _build_program(parity: int):
    import concourse.mybir as mybir
    import concourse.tile as tile
    from concourse import bacc
    from concourse.masks import make_identity

    f32 = mybir.dt.float32
    bf16 = mybir.dt.bfloat16

    nc = bacc.Bacc("TRN2", target_bir_lowering=False, debug=False)
    x = nc.dram_tensor("x", [T, C], f32, kind="ExternalInput").ap()
    w_qkv = nc.dram_tensor("w_qkv", [3 * C, C], f32, kind="ExternalInput").ap()
    w_out = nc.dram_tensor("w_out", [C, C], f32, kind="ExternalInput").ap()
    y = nc.dram_tensor("y", [TL, C], f32, kind="ExternalOutput").ap()

    g512 = QMAP512[parity]
    scale = 1.0 / float(np.sqrt(DH))

    with tile.TileContext(nc) as tc:
        with (
            tc.tile_pool(name="res", bufs=1) as res,
            tc.tile_pool(name="stage", bufs=3) as stage,
            tc.tile_pool(name="wtile", bufs=2) as wtile,
            tc.tile_pool(name="work", bufs=4) as work,
            tc.tile_pool(name="rdp", bufs=2) as rdp,
            tc.tile_pool(name="attn", bufs=2) as attnp,
        ):
            ones128 = res.tile([P, P], bf16)
            nc.vector.memset(ones128, 1.0)

            # [128,128] triangle mask: keep (1.0) iff q-col >= key-row
            tri = res.tile([P, P], bf16)
            nc.gpsimd.memset(tri, 1.0)
            nc.gpsimd.affine_select(
                out=tri, in_=tri, compare_op=mybir.AluOpType.is_ge,
                fill=0.0, base=0, pattern=[[1, P]], channel_multiplier=-1,
            )

            ident = res.tile([P, P], bf16)
            make_identity(nc, ident)

            # ---- residents
            kT = res.tile([P, NPAIR, T], bf16)          # K^T   4 MB
            qT = res.tile([P, NPAIR, TL], bf16)         # Q^T   2 MB
            v = res.tile([P, T // P, C], bf16)          # V     4 MB
            wvT = res.tile([P, NCT, C], bf16)           # w_v^T 2 MB
            woT = res.tile([P, NCT, C], bf16)           # w_out^T 2 MB
            xT = res.tile([P, NCT, T], bf16)            # x^T   4 MB

            _cp = [0]

            def rr_copy(out, in_):
                if _cp[0] % 2 == 0:
                    nc.scalar.copy(out=out, in_=in_)
                else:
                    nc.vector.tensor_copy(out=out, in_=in_)
                _cp[0] += 1

            def load_cast(src_ap):
                lf = stage.tile([P, C], f32, tag="ldf")
                nc.sync.dma_start(out=lf, in_=src_ap)
                lb = stage.tile([P, C], bf16, tag="ldb")
                nc.any.tensor_copy(out=lb, in_=lf)
                return lb

            def make_transpose_block(pool):
                def transpose_block(lb, dst, dst_col):
                    for half in range(2):
                        pt4 = pool.tile([P, 4, P], bf16, tag="pt4",
                                        name="pt4")
                        for q in range(4):
                            ct = half * 4 + q
                            nc.tensor.transpose(
                                pt4[:, q, :], lb[:, ct * P:(ct + 1) * P],
                                ident)
                        rr_copy(
                            out=dst[:, half * 4:(half + 1) * 4,
                                    dst_col:dst_col + P],
                            in_=pt4)
                return transpose_block

            # ====== phases A+B: transposes + K/Q projections ======
            with (
                tc.tile_pool(name="pst", bufs=4, space="PSUM") as pst,
                tc.tile_pool(name="psqkv", bufs=4, space="PSUM") as psqkv,
            ):
                transpose_block = make_transpose_block(pst)

                # A: x^T, interleaved with the w_v / w_out block transposes
                for tt in range(T // P):
                    xb = load_cast(x[tt * P:(tt + 1) * P, :])
                    transpose_block(xb, xT, tt * P)
                    if tt < 8:
                        wb = load_cast(w_qkv[(16 + tt) * P:(17 + tt) * P, :])
                        transpose_block(wb, wvT, tt * P)
                    else:
                        fb = tt - 8
                        wb = load_cast(w_out[fb * P:(fb + 1) * P, :])
                        transpose_block(wb, woT, fb * P)

                # B: Q/K projections (Q: only local halves)
                for fb in range(16):                     # 0..7 Q, 8..15 K
                    wb = load_cast(w_qkv[fb * P:(fb + 1) * P, :])
                    wqk = wtile.tile([P, NCT, P], bf16, tag="wqk")
                    transpose_block(wb, wqk, 0)
                    if fb < 8:
                        for u in range(NQT):
                            ps = psqkv.tile([P, 512], f32, tag="psqkv")
                            t0 = g512[u] * 512
                            for ct in range(NCT):
                                nc.tensor.matmul(
                                    ps, wqk[:, ct, :],
                                    xT[:, ct, t0:t0 + 512],
                                    start=(ct == 0), stop=(ct == NCT - 1),
                                )
                            nc.vector.tensor_copy(
                                out=qT[:, fb, u * 512:(u + 1) * 512], in_=ps)
                    else:
                        pr = fb - 8
                        for u in range(4):
                            ps = psqkv.tile([P, 512], f32, tag="psqkv")
                            for ct in range(NCT):
                                nc.tensor.matmul(
                                    ps, wqk[:, ct, :],
                                    xT[:, ct, u * 512:(u + 1) * 512],
                                    start=(ct == 0), stop=(ct == NCT - 1),
                                )
                            nc.vector.tensor_copy(
                                out=kT[:, pr, u * 512:(u + 1) * 512], in_=ps)

            # ====== phases C-F: V projection + attention + out-proj ======
            with (
                tc.tile_pool(name="psv", bufs=2, space="PSUM") as psv,
                tc.tile_pool(name="pss", bufs=2, space="PSUM") as pss,
                tc.tile_pool(name="pso", bufs=2, space="PSUM") as pso,
            ):
                def v_step(tt, fo):
                    ps = psv.tile([P, 512], f32, tag="psv")
                    for ct in range(NCT):
                        nc.tensor.matmul(
                            ps, xT[:, ct, tt * P:(tt + 1) * P],
                            wvT[:, ct, fo * 512:(fo + 1) * 512],
                            start=(ct == 0), stop=(ct == NCT - 1),
                        )
                    nc.vector.tensor_copy(
                        out=v[:, tt, fo * 512:(fo + 1) * 512], in_=ps)

                attnT = [
                    attnp.tile([P, NPAIR, QT_TILE], bf16, tag="attnT",
                               name=f"attnT{j}")
                    for j in range(NQT)
                ]

                def attn_pair_steps(p, j):
                    """List of emission steps for attention of (pair, qtile).
                    Scores for k-tile k are emitted together with the
                    softmax+PV of k-tile k-1 so the PE never queues directly
                    behind the ACT engine's exp."""
                    G = g512[j]
                    nk = 4 * (G + 1)
                    box = {}
                    qA = qT[0:64, p, j * QT_TILE:(j + 1) * QT_TILE]
                    qB = qT[64:128, p, j * QT_TILE:(j + 1) * QT_TILE]

                    def softmax_pv(k, s2, qlo):
                        dj = k - 4 * G
                        p2 = work.tile([P, 2, QT_TILE], bf16, tag="p2")
                        nc.scalar.activation(
                            p2[:, :, qlo:], s2[:, :, qlo:],
                            mybir.ActivationFunctionType.Exp, scale=scale)
                        if dj >= 0:  # triangle block on the diagonal
                            nc.vector.tensor_mul(
                                out=p2[:, :, qlo:qlo + P],
                                in0=p2[:, :, qlo:qlo + P],
                                in1=tri[:, None, :].to_broadcast((P, 2, P)))
                        pA = p2[:, 0, qlo:]
                        pB = p2[:, 1, qlo:]
                        st, sp_ = (k == 0), (k == nk - 1)
                        nc.tensor.matmul(box["pdA"][:, qlo:],
                                         ones128[:, 0:64], pA,
                                         start=st, stop=sp_,
                                         skip_group_check=True)
                        nc.tensor.matmul(box["pdB"][:, qlo:],
                                         ones128[:, 0:64], pB,
                                         start=st, stop=sp_,
                                         skip_group_check=True)
                        nc.tensor.matmul(
                            box["poA"][:, qlo:], v[:, k, p * P:p * P + 64],
                            pA, start=st, stop=sp_, skip_group_check=True)
                        nc.tensor.matmul(
                            box["poB"][:, qlo:],
                            v[:, k, p * P + 64:(p + 1) * P], pB,
                            start=st, stop=sp_, skip_group_check=True)

                    steps = []

                    def mk_ktile(k):
                        dj = k - 4 * G
                        qlo = max(dj, 0) * P

                        def s():
                            if k == 0:
                                poA_t = pso.tile([P, QT_TILE], f32,
                                                 tag="po", name="poA")
                                poB_t = pso.tile([P, QT_TILE], f32,
                                                 tag="po", name="poB")
                                box["poA"] = poA_t[0:64]
                                box["poB"] = poB_t[64:128]
                                box["pdA"] = poB_t[0:64]
                                box["pdB"] = poA_t[64:128]
                            s2 = pss.tile([P, 2, QT_TILE], f32, tag="s2")
                            ks = slice(k * P, (k + 1) * P)
                            nc.tensor.matmul(s2[:, 0, qlo:],
                                             kT[0:64, p, ks], qA[:, qlo:])
                            nc.tensor.matmul(s2[:, 1, qlo:],
                                             kT[64:128, p, ks], qB[:, qlo:])
                            if box.get("pending") is not None:
                                softmax_pv(*box["pending"])
                            box["pending"] = (k, s2, qlo)
                        return s
                    steps += [mk_ktile(k) for k in range(nk)]

                    def s_finish():
                        softmax_pv(*box["pending"])
                        # reciprocal_approx_fast mangles PSUM reads on HW —
                        # stage the denominators through SBUF first.
                        dcp = rdp.tile([P, QT_TILE], f32, tag="dcp")
                        nc.vector.tensor_copy(out=dcp[0:64], in_=box["pdA"])
                        nc.vector.tensor_copy(out=dcp[64:128],
                                              in_=box["pdB"])
                        rD = rdp.tile([P, QT_TILE], f32, tag="rD")
                        nc.vector.reciprocal_approx_fast(rD, dcp)
                        nc.vector.tensor_mul(
                            out=attnT[j][0:64, p, :], in0=box["poA"],
                            in1=rD[0:64])
                        nc.vector.tensor_mul(
                            out=attnT[j][64:128, p, :], in0=box["poB"],
                            in1=rD[64:128])
                    steps.append(s_finish)
                    return steps

                def outproj_chunk(j, sub, fo):
                    qs = slice(sub * P, (sub + 1) * P)
                    ps = pso.tile([P, 512], f32, tag="po", name="psy")
                    for p in range(NPAIR):
                        nc.tensor.matmul(
                            ps, attnT[j][:, p, qs],
                            woT[:, p, fo * 512:(fo + 1) * 512],
                            start=(p == 0), stop=(p == NPAIR - 1),
                        )
                    ysb = stage.tile([P, 512], f32, tag="ysb")
                    # Vector-pinned: Scalar is the exp pacer during j=1
                    nc.vector.tensor_copy(out=ysb, in_=ps)
                    nc.sync.dma_start(
                        out=y[j * QT_TILE + sub * P:
                              j * QT_TILE + (sub + 1) * P,
                              fo * 512:(fo + 1) * 512],
                        in_=ysb)

                # C: V tiles needed by the j=0 attention (keys 0..512*(G0+1))
                tt_first = 4 * (g512[0] + 1)
                for tt in range(tt_first):
                    for fo in range(2):
                        v_step(tt, fo)

                # D: remaining V tiles interleaved with the j=0 attention
                vrest = [(tt, fo) for tt in range(tt_first, T // P)
                         for fo in range(2)]
                asteps = []
                for p in range(NPAIR):
                    asteps += attn_pair_steps(p, 0)
                rate = len(asteps) / max(len(vrest), 1)
                credit, qi = 0.0, 0
                for tt, fo in vrest:
                    v_step(tt, fo)
                    credit += rate
                    while credit >= 1.0 and qi < len(asteps):
                        asteps[qi]()
                        qi += 1
                        credit -= 1.0
                while qi < len(asteps):
                    asteps[qi]()
                    qi += 1

                # E: j=1 attention, cross-pair software-pipelined: pair p's
                # finish + the j=0 out-proj chunk are emitted after pair
                # p+1's first scores, so the ACT exp stream never drains at
                # a pair boundary.
                oj0 = [(0, sub, fo) for sub in range(QT_TILE // P)
                       for fo in range(2)]
                prev_tail = []
                for p in range(NPAIR):
                    ks = attn_pair_steps(p, 1)
                    ks[0]()
                    for s in prev_tail:
                        s()
                    for s in ks[1:-1]:
                        s()
                    prev_tail = [ks[-1],
                                 (lambda p=p: outproj_chunk(*oj0[p]))]
                for s in prev_tail:
                    s()

                # F: j=1 out-proj
                for sub in range(QT_TILE // P):
                    for fo in range(2):
                        outproj_chunk(1, sub, fo)

    nc.compile()
    return nc


def _get_program(parity: int):
    if parity not in _cache:
        _cache[parity] = _build_program(parity)
    return _cache[parity]


def _run_group(nc, in_maps, devices, out_holder, idx):
    """shard_map the program over `devices`, one in_map per device."""
    import jax
    from jax.sharding import Mesh, PartitionSpec
    from jax.experimental.shard_map import shard_map
    import concourse.mybir as mybir
    from concourse.bass2jax import (
        _bass_exec_p, install_neuronx_cc_hook, partition_id_tensor)

    install_neuronx_cc_hook()

    partition_name = (
        nc.partition_id_tensor.name if nc.partition_id_tensor else None)
    in_names, out_names, out_avals, zero_outs = [], [], [], []
    for alloc in nc.m.functions[0].allocations:
        if not isinstance(alloc, mybir.MemoryLocationSet):
            continue
        name = alloc.memorylocations[0].name
        if alloc.kind == "ExternalInput":
            if name != partition_name:
                in_names.append(name)
        elif alloc.kind == "ExternalOutput":
            out_names.append(name)
            shape = tuple(alloc.tensor_shape)
            dtype = mybir.dt.np(alloc.dtype)
            out_avals.append(jax.core.ShapedArray(shape, dtype))
            zero_outs.append(np.zeros(shape, dtype))
    n_params = len(in_names)
    n_outs = len(out_avals)
    all_names = in_names + out_names
    if partition_name is not None:
        all_names.append(partition_name)
    donate = tuple(range(n_params, n_params + n_outs))

    def _body(*args):
        operands = list(args)
        if partition_name is not None:
            operands.append(partition_id_tensor())
        outs = _bass_exec_p.bind(
            *operands,
            out_avals=tuple(out_avals),
            in_names=tuple(all_names),
            out_names=tuple(out_names),
            lowering_input_output_aliases=(),
            sim_require_finite=False,
            sim_require_nnan=False,
            nc=nc,
        )
        return tuple(outs)

    n = len(devices)
    mesh = Mesh(np.asarray(devices), ("core",))
    sharded = jax.jit(
        shard_map(
            _body, mesh=mesh,
            in_specs=(PartitionSpec("core"),) * (n_params + n_outs),
            out_specs=(PartitionSpec("core"),) * n_outs,
            check_rep=False,
        ),
        donate_argnums=donate, keep_unused=True,
    )
    concat_in = [
        np.concatenate([np.asarray(m[name]) for m in in_maps], axis=0)
        for name in in_names
    ]
    concat_zero = [
        np.zeros((n * z.shape[0], *z.shape[1:]), z.dtype) for z in zero_outs
    ]
    out_arrs = sharded(*concat_in, *concat_zero)
    out_holder[idx] = [
        {
            name: np.asarray(out_arrs[i]).reshape(n, *out_avals[i].shape)[c]
            for i, name in enumerate(out_names)
        }
        for c in range(n)
    ]


def kernel(x, attn_mask, w_qkv, w_out):
    """Full inputs in, full output out. attn_mask is all-ones (per the
    problem spec) so masking reduces to the causal structure."""
    import jax

    x = np.asarray(x, dtype=np.float32)
    w_qkv = np.asarray(w_qkv, dtype=np.float32)
    w_out = np.asarray(w_out, dtype=np.float32)

    nc_e = _get_program(0)
    nc_o = _get_program(1)

    devices = jax.devices()
    in_maps = [
        {"x": x[b], "w_qkv": w_qkv, "w_out": w_out} for b in range(B)
    ]

    results = [None, None]
    t_e = threading.Thread(
        target=_run_group, args=(nc_e, in_maps, devices[0:4], results, 0))
    t_o = threading.Thread(
        target=_run_group, args=(nc_o, in_maps, devices[4:8], results, 1))
    t_e.start(); t_o.start()
    t_e.join(); t_o.join()

    y = np.empty((B, T, C), dtype=np.float32)
    for parity, group in enumerate(results):
        for b in range(B):
            y_local = group[b]["y"]          # [TL, C] in local q order
            for j in range(NQT):
                G = QMAP512[parity][j]
                y[b, G * QT_TILE:(G + 1) * QT_TILE, :] = \
                    y_local[j * QT_TILE:(j + 1) * QT_TILE, :]
    return y
